# revision 16
# baseline (speedup 1.0000x reference)
"""v6.2: bf16 implicit-GEMM conv (FWL weight loads), strided-rhs, t-outer.

vs v6: no scratch warmup — first real matmuls run cold (~1.2 GHz) for
~3.4us (+1.7us penalty) but start ~6us earlier. All DMAs stay on the
one sync HWDGE queue in strict priority order (parallel queues just
share HBM bandwidth and delay the critical piece): img0 rows 0-9,
h0-half of weights, img0 rows 8-25, h1 weights, bias, img0 rows 24-57.
First matmul is gated by ~530KB instead of ~1.5MB.
"""

import sys

if "/opt/trn_rl_repo" not in sys.path:
    sys.path.insert(0, "/opt/trn_rl_repo")

import numpy as np

N, C_IN, H, W = 32, 128, 56, 56
C_OUT, KH, KW = 256, 3, 3
N_CORES = 8
IMGS = N // N_CORES
HP, WP = H + 2, W + 2
RPT = 8
NT = H // RPT          # 7
TF = RPT * W           # 448
NH = C_OUT // 128      # 2

XA_R0, XA_R1 = 0, 26   # rows for tiles t=0..2
XB_R0, XB_R1 = 24, 58  # rows for tiles t=3..6
T_SPLIT = 3
N_WARMUP_MM = 8        # fills the PE during the first DMA wait; HAM warm
                       # needs ~3.4us of busy before real matmuls land

_CACHE = {}


def _build_program():
    import concourse.mybir as mybir
    import concourse.tile as tile
    from concourse import bacc

    F32 = mybir.dt.float32
    BF16 = mybir.dt.bfloat16

    nc = bacc.Bacc("TRN2", target_bir_lowering=False, debug=False,
                   enable_asserts=False)

    xp = nc.dram_tensor("xp", [IMGS, C_IN, HP, WP], BF16,
                        kind="ExternalInput").ap()
    w = nc.dram_tensor("w", [NH, C_IN, KH * KW, 128], BF16,
                       kind="ExternalInput").ap()
    b = nc.dram_tensor("b", [128, NH], F32, kind="ExternalInput").ap()
    out = nc.dram_tensor("out", [IMGS, C_OUT, H, W], F32,
                         kind="ExternalOutput").ap()
    out_v = out.rearrange("n c a b -> n c (a b)")

    with tile.TileContext(nc) as tc:
        with (
            tc.tile_pool(name="consts", bufs=1) as consts,
            tc.tile_pool(name="xin", bufs=1) as xin,
            tc.tile_pool(name="outp", bufs=2) as outp,
            tc.tile_pool(name="psum", bufs=7, space="PSUM") as psum,
        ):
            w_sb = consts.tile([C_IN, NH, KH * KW, 128], BF16, tag="w")
            b_sb = consts.tile([128, NH], F32, tag="b")

            # img0 front slab split so the first matmul is gated by the
            # smallest possible prefix: rows 0-9 (t=0, sync queue) + h0
            # weights (scalar queue, issued in parallel).
            xa1 = xin.tile([C_IN, 10, WP], BF16, tag="xa1", bufs=1)
            nc.sync.dma_start(out=xa1[:], in_=xp[0, :, 0:10])
            nc.scalar.dma_start(out=w_sb[:, 0], in_=w[0])
            xa2 = xin.tile([C_IN, 18, WP], BF16, tag="xa2", bufs=1)
            nc.sync.dma_start(out=xa2[:], in_=xp[0, :, 8:26])
            nc.scalar.dma_start(out=w_sb[:, 1], in_=w[1])
            nc.scalar.dma_start(out=b_sb[:], in_=b)
            xb0 = xin.tile([C_IN, XB_R1 - XB_R0, WP], BF16, tag="xb")
            nc.sync.dma_start(out=xb0[:], in_=xp[0, :, XB_R0:XB_R1])

            scratch = consts.tile([128, TF], BF16, tag="scratch")
            nc.gpsimd.memset(scratch[:], 0.0)
            warm_ps = psum.tile([128, TF], F32, tag="warm", bufs=1)
            for _ in range(N_WARMUP_MM):
                nc.tensor.matmul(warm_ps[:, :], lhsT=scratch[:, :128],
                                 rhs=scratch[:, :], start=True, stop=True)

            xts = {0: (xa1, xa2, xb0)}
            for img in range(1, IMGS):
                xa = xin.tile([C_IN, XA_R1 - XA_R0, WP], BF16, tag="xa")
                nc.sync.dma_start(out=xa[:], in_=xp[img, :, XA_R0:XA_R1])
                xb = xin.tile([C_IN, XB_R1 - XB_R0, WP], BF16, tag="xb")
                nc.sync.dma_start(out=xb[:], in_=xp[img, :, XB_R0:XB_R1])
                xts[img] = (xa, xb)

            for img in range(IMGS):
                ots = [outp.tile([128, H * W], F32, tag=f"ot{h}",
                                 name=f"ot{img}_{h}")
                       for h in range(NH)]
                # img0 runs h-outer so the h1 weight DMA (arrives ~3us
                # after h0) is never on the critical path
                if img == 0:
                    order = [(t, h) for h in range(NH) for t in range(NT)]
                else:
                    order = [(t, h) for t in range(NT) for h in range(NH)]
                for t, h in order:
                    if img == 0:
                        if t == 0:
                            src, r_off = xts[0][0], 0
                        elif t < T_SPLIT:
                            src, r_off = xts[0][1], 8
                        else:
                            src, r_off = xts[0][2], XB_R0
                    elif t < T_SPLIT:
                        src, r_off = xts[img][0], XA_R0
                    else:
                        src, r_off = xts[img][1], XB_R0
                    r0 = RPT * t - r_off
                    pt = psum.tile([128, TF], F32, tag="pt")
                    for k in range(KH * KW):
                        kh, kw = divmod(k, KW)
                        nc.tensor.matmul(
                            pt[:, :],
                            lhsT=w_sb[:, h, k],
                            rhs=src[:, r0 + kh:r0 + kh + RPT, kw:kw + W],
                            start=(k == 0),
                            stop=(k == KH * KW - 1),
                        )
                    nc.vector.tensor_scalar_add(
                        out=ots[h][:, t * TF:(t + 1) * TF],
                        in0=pt[:, :],
                        scalar1=b_sb[:, h:h + 1],
                    )
                    nc.sync.dma_start(
                        out=out_v[img, h * 128:(h + 1) * 128,
                                  t * TF:(t + 1) * TF],
                        in_=ots[h][:, t * TF:(t + 1) * TF])
    nc.compile()
    return nc


def get_program():
    if "nc" not in _CACHE:
        _CACHE["nc"] = _build_program()
    return _CACHE["nc"]


def make_in_maps(x, weight, bias):
    import ml_dtypes

    BF = ml_dtypes.bfloat16
    x = np.asarray(x)
    weight = np.asarray(weight)
    bias = np.asarray(bias, dtype=np.float32)

    xpad = np.zeros((N, C_IN, HP, WP), dtype=BF)
    xpad[:, :, 1:1 + H, 1:1 + W] = x.astype(BF)
    # [NH, C_in, kh*kw, 128]: h-major so each half is one contiguous DMA
    w_t = np.ascontiguousarray(
        weight.astype(np.float32).transpose(1, 2, 3, 0)
        .reshape(C_IN, KH * KW, NH, 128).transpose(2, 0, 1, 3).astype(BF))
    b2 = np.ascontiguousarray(bias.reshape(NH, 128).T)

    return [
        {
            "xp": np.ascontiguousarray(xpad[i * IMGS:(i + 1) * IMGS]),
            "w": w_t,
            "b": b2,
        }
        for i in range(N_CORES)
    ]


def kernel(x, weight, bias):
    from concourse.bass_utils import run_bass_kernel_spmd

    nc = get_program()
    in_maps = make_in_maps(x, weight, bias)
    res = run_bass_kernel_spmd(nc, in_maps, core_ids=list(range(N_CORES)))
    return np.concatenate([res.results[i]["out"] for i in range(N_CORES)],
                          axis=0)
